# revision 47
# baseline (speedup 1.0000x reference)
"""Trainium2 Bass kernel for nn_LogicDense (difflogic dense layer).

Math (reference):
    w      = softmax(weight, axis=-1)            # [out_dim, 16]
    coeffs = w @ GATE_COEFFS                     # [out_dim, 4] = (c0, ca, cb, cab)
    a      = x[:, indices[0]]                    # [batch, out_dim]
    b      = x[:, indices[1]]
    out    = c0 + ca*a + cb*b + cab*a*b          # [batch, out_dim]

Strategy (8 NeuronCores, tensor-parallel over out_dim):
    - Core c owns output rows j in [2048*c, 2048*(c+1)), processed in 16
      chunks of 128 rows (one output row per SBUF partition).
    - Host preps: x_t [in_dim, batch] as fp16 and as u8 (sa = round(250*x));
      per-core chunk indices [128, 16] i32; softmax+gate-coeff collapse
      (O(out_dim*16) weight prep) folded into a [128, 64] coeff tile.
    - Per chunk: two DGE indirect DMAs (indirect_dma_start, per-partition
      row offsets -- no Q7 ucode library, so no ~10us library-load stall
      and near-zero issue cost) gather a-rows (u8) and b-rows (fp16).
    - The DMA engines drain the qPoolDynamic ring FIFO, so gather issue is
      PACED to DEPTH chunks in flight: without it the DGE prefills the
      ring, the interleaved stores land tens of us late, and compute
      stalls on o-buf reuse (that failure mode measured +30us). Stores
      lag issue by ST_LAG chunks and NO=7 o-bufs absorb the remaining
      store-completion latency.
    - All 16 gates map [0,1]^2 -> [0,1] and softmax weights are convex, so
      out is in [0,1]. Computed as out_u = 250*out + 2.5 (in [2.5, 252.5])
      and the store DMA (gpsimd -- only SWDGE casts) converts fp16 -> u8,
      halving store traffic. Host decodes (u - 2.5)/250.
    - Per-partition coeff scalars give a 4-instruction combine:
         t = cab*sa + 250*cb           (tensor_scalar / ACT Identity)
         h = ca*sa + (250*c0+2.5)      (ACT Identity, scale/bias APs)
         o = t * b                     (DVE tensor_mul, fp16 2x)
         o = o + h                     (DVE tensor_add, fp16 2x)
      The ts op alternates ACT/DVE per chunk to balance engines
      (DVE ~90us, ACT ~90us busy; DMA ~97us busy at ~345 GB/s is the
      roofline). A dummy ACT op hoists the act-table load off the first
      data-dependent activation.
    - HBM/core: 8 MiB a(u8) + 16 MiB b(fp16) + 8 MiB store(u8) = 32 MiB;
      b must stay fp16: DVE tensor_tensor only reaches 2x with 2-byte
      operands, and a u8 mul at 1x would make DVE the bottleneck.
"""

import os
import sys

import numpy as np

sys.path.insert(0, "/opt/trn_rl_repo")

BATCH = 4096
IN_DIM = 8192
OUT_DIM = 16384
N_CORES = 8
J_SHARD = OUT_DIM // N_CORES        # 2048 output rows per core
CHUNK = 128                         # output rows per pipeline iteration
N_CHUNKS = J_SHARD // CHUNK         # 16
NG = 6                              # gather buffer slots (a and b paired)
NT = 2                              # t buffer sets (ts -> mul lifetime)
NH = 3                              # h buffer sets (h -> add lifetime)
NO = 7                              # output buffer sets (add -> store)
ST_LAG = 6                          # store issue lag behind gather issue
DEPTH = 4                           # max gather chunks in the DMA ring

OUT_SCALE = 250.0                   # out_u = OUT_SCALE*out + OUT_OFF
OUT_OFF = 2.5

GATE_COEFFS = np.array([
    [0, 0, 0, 0], [0, 0, 0, 1], [0, 1, 0, -1], [0, 1, 0, 0],
    [0, 0, 1, -1], [0, 0, 1, 0], [0, 1, 1, -2], [0, 1, 1, -1],
    [1, -1, -1, 1], [1, -1, -1, 2], [1, 0, -1, 0], [1, 0, -1, 1],
    [1, -1, 0, 0], [1, -1, 0, 1], [1, 0, 0, -1], [1, 0, 0, 0],
], dtype=np.float32)                # [16 gates, 4 bilinear coeffs]

_CACHE = {}
LAST_RESULT = None  # BassKernelResults of the most recent run (for profiling)


def _build_program():
    import concourse.bacc as bacc
    import concourse.bass as bass
    import concourse.mybir as mybir
    from contextlib import ExitStack

    dt = mybir.dt
    AF = mybir.ActivationFunctionType

    nc = bacc.Bacc("TRN2", target_bir_lowering=False, debug=False)

    xt16 = nc.dram_tensor("xt16", [IN_DIM, BATCH], dt.float16,
                          kind="ExternalInput")
    xt8 = nc.dram_tensor("xt8", [IN_DIM, BATCH], dt.uint8,
                         kind="ExternalInput")
    # cols [0,16): a-row index for chunk i; [16,32): b-row index
    idx = nc.dram_tensor("idx", [128, 2 * N_CHUNKS], dt.int32,
                         kind="ExternalInput")
    # [:, 16*k + i] = coeff k (0=c0,1=ca,2=cb,3=cab) for chunk i
    cct = nc.dram_tensor("cct", [128, 4 * N_CHUNKS], dt.float32,
                         kind="ExternalInput")
    out = nc.dram_tensor("out", [J_SHARD, BATCH], dt.uint8,
                         kind="ExternalOutput")

    with ExitStack() as ctx:
        sb = lambda name, shape, dty: ctx.enter_context(
            nc.sbuf_tensor(name, shape, dty))
        sb_idx = sb("sb_idx", [128, 2 * N_CHUNKS], dt.int32)
        sb_cc = sb("sb_cc", [128, 4 * N_CHUNKS], dt.float32)
        a_bufs = [sb(f"a{k}", [128, BATCH], dt.uint8) for k in range(NG)]
        b_bufs = [sb(f"b{k}", [128, BATCH], dt.float16) for k in range(NG)]
        t_bufs = [sb(f"t{k}", [128, BATCH], dt.float16) for k in range(NT)]
        h_bufs = [sb(f"h{k}", [128, BATCH], dt.float16) for k in range(NH)]
        o_bufs = [sb(f"o{k}", [128, BATCH], dt.float16) for k in range(NO)]

        # ts(i): even chunks on ACT, odd on DVE (tensor_scalar, u8 2x_2P).
        ts_on_act = lambda i: i % 2 == 0

        ops_act = []
        for i in range(N_CHUNKS):
            if ts_on_act(i):
                ops_act.append(('t', i))
            ops_act.append(('h', i))
        act_val = {op: n + 1 for n, op in enumerate(ops_act)}

        ops_dve = []
        for i in range(N_CHUNKS):
            if not ts_on_act(i):
                ops_dve.append(('ts', i))
            if i > 0:
                ops_dve.append(('add', i - 1))
            ops_dve.append(('mul', i))
        ops_dve.append(('add', N_CHUNKS - 1))
        dve_val = {op: n + 1 for n, op in enumerate(ops_dve)}

        sem = lambda name: ctx.enter_context(nc.semaphore(name))
        s_pi = sem("s_pi")
        s_pc = sem("s_pc")
        # slot k = chunk i % NG; a-gather of chunk i incs s_a[k] by 16,
        # b-gather incs s_g[k] by 16 (completion order between the two
        # DMAs is NOT guaranteed, hence separate semaphores).
        s_g = [sem(f"s_g{k}") for k in range(NG)]
        s_a = [sem(f"s_a{k}") for k in range(NG)]
        s_st = [sem(f"s_st{k}") for k in range(NO)]
        s_act = sem("s_act")
        s_dve = sem("s_dve")

        with nc.Block() as block:

            def cseg(k, i):  # per-partition scalar AP: coeff k, chunk i
                return sb_cc[:, 16 * k + i : 16 * k + i + 1]

            @block.sync
            def _(sync):
                sync.dma_start(sb_cc[:, :], cct[:, :]).then_inc(s_pc, 16)

            @block.gpsimd
            def _(gp):
                # load idx from the gp queue itself: gp starts ~2us before
                # the sync engine's first DMA reaches the queues.
                gp.dma_start(sb_idx[:, :], idx[:, :]).then_inc(s_pi, 16)
                gp.wait_ge(s_pi, 16)  # idx tile loaded

                def igather(dst, src, col):
                    return gp.indirect_dma_start(
                        out=dst, out_offset=None,
                        in_=src,
                        in_offset=bass.IndirectOffsetOnAxis(
                            ap=sb_idx[:, col:col + 1], axis=0),
                    )

                def store(i):  # store chunk i (SWDGE cast fp16 -> u8)
                    ko = i % NO
                    gp.wait_ge(s_dve, dve_val[('add', i)])
                    if i >= NO:
                        gp.wait_ge(s_st[ko], 16 * (i // NO))
                    gp.dma_start(out[i * CHUNK:(i + 1) * CHUNK, :],
                                 o_bufs[ko][:, :]).then_inc(s_st[ko], 16)

                for i in range(N_CHUNKS):
                    k = i % NG
                    if i >= DEPTH:
                        # ring pacing: the DMA engines drain descriptors
                        # FIFO, so cap the gather backlog at DEPTH chunks --
                        # otherwise interleaved stores sit behind tens of us
                        # of gather bytes and compute stalls on o-buf reuse.
                        kp = (i - DEPTH) % NG
                        gp.wait_ge(s_g[kp], 16 * ((i - DEPTH) // NG + 1))
                    if i >= NG:
                        last = i - NG
                        # slot free once ts+h+mul of chunk last consumed it
                        # (mul(last) implies ts(last) on either engine).
                        gp.wait_ge(s_act, act_val[('h', last)])
                        gp.wait_ge(s_dve, dve_val[('mul', last)])
                        gp.wait_ge(s_a[k], 16 * (i // NG))
                        gp.wait_ge(s_g[k], 16 * (i // NG))
                    igather(a_bufs[k][:, :], xt8[:, :], i
                            ).then_inc(s_a[k], 16)
                    igather(b_bufs[k][:, :], xt16[:, :], N_CHUNKS + i
                            ).then_inc(s_g[k], 16)
                    # stores lag gather issue by ST_LAG chunks; add(i-ST_LAG)
                    # completes before gather i issues (compute trails the
                    # paced gather stream), so the gp queue never stalls.
                    if i >= ST_LAG:
                        store(i - ST_LAG)
                for i in range(N_CHUNKS - ST_LAG, N_CHUNKS):
                    store(i)
                for ko in range(NO):
                    if ko <= N_CHUNKS - 1:
                        n_st = (N_CHUNKS - 1 - ko) // NO + 1
                        gp.wait_ge(s_st[ko], 16 * n_st)

            @block.scalar
            def _(sc):
                sc.wait_ge(s_pc, 16)  # coeff tile ready
                # warmup op: hoists the auto-inserted ACT_TABLE_LOAD off the
                # first real activation (which waits on gather data).
                sc.activation(t_bufs[0][:, :1], sb_cc[:, :1], AF.Identity)
                for kind, i in ops_act:
                    ka = i % NG
                    sc.wait_ge(s_a[ka], 16 * (i // NG + 1))  # a(i) landed
                    if kind == 't':
                        kt = i % NT
                        # t[kt] free once DVE mul of i-NT consumed it
                        if i >= NT:
                            sc.wait_ge(s_dve, dve_val[('mul', i - NT)])
                        # t = cab*sa + 250*cb  (u8 source read directly)
                        sc.activation(t_bufs[kt][:, :], a_bufs[ka][:, :],
                                      AF.Identity,
                                      bias=cseg(2, i), scale=cseg(3, i),
                                      ).then_inc(s_act, 1)
                    else:
                        kh = i % NH
                        # h[kh] free once DVE add of i-NH completed
                        if i >= NH:
                            sc.wait_ge(s_dve, dve_val[('add', i - NH)])
                        # h = ca*sa + (250*c0 + 2.5)
                        sc.activation(h_bufs[kh][:, :], a_bufs[ka][:, :],
                                      AF.Identity,
                                      bias=cseg(0, i), scale=cseg(1, i),
                                      ).then_inc(s_act, 1)

            @block.vector
            def _(v):
                MU, AD = mybir.AluOpType.mult, mybir.AluOpType.add
                v.wait_ge(s_pc, 16)  # coeff tile ready
                for kind, i in ops_dve:
                    kt, kh = i % NT, i % NH
                    ko = i % NO
                    ka = i % NG
                    if kind == 'ts':
                        # t = (sa * cab) + 250*cb  (u8 src, 2x_2P mode)
                        v.wait_ge(s_a[ka], 16 * (i // NG + 1))
                        if i >= NT:
                            # t[kt] free once mul of chunk i-NT consumed it
                            v.wait_ge(s_dve, dve_val[('mul', i - NT)])
                        v.tensor_scalar(t_bufs[kt][:, :], a_bufs[ka][:, :],
                                        cseg(3, i), cseg(2, i), MU, AD,
                                        ).then_inc(s_dve, 1)
                    elif kind == 'mul':
                        v.wait_ge(s_g[ka], 16 * (i // NG + 1))  # b(i) landed
                        if ts_on_act(i):
                            v.wait_ge(s_act, act_val[('t', i)])
                        else:
                            v.wait_ge(s_dve, dve_val[('ts', i)])
                        if i >= NO:
                            # o[ko] free once store of chunk i-NO completed
                            v.wait_ge(s_st[ko], 16 * (i // NO))
                        v.tensor_mul(o_bufs[ko][:, :], t_bufs[kt][:, :],
                                     b_bufs[ka][:, :]).then_inc(s_dve, 1)
                    else:  # add
                        v.wait_ge(s_act, act_val[('h', i)])
                        v.wait_ge(s_dve, dve_val[('mul', i)])
                        v.tensor_add(o_bufs[ko][:, :],
                                     o_bufs[ko][:, :],
                                     h_bufs[kh][:, :]).then_inc(s_dve, 1)

    nc.compile()
    return nc


def _get_program():
    if "nc" not in _CACHE:
        _CACHE["nc"] = _build_program()
    return _CACHE["nc"]


def kernel(x, weight, indices):
    global LAST_RESULT
    from concourse.bass_utils import run_bass_kernel_spmd

    x = np.asarray(x, dtype=np.float32)
    weight = np.asarray(weight, dtype=np.float32)
    indices = np.asarray(indices)

    nc = _get_program()

    xt = np.ascontiguousarray(x.T)                       # [in_dim, batch] f32
    xt16 = xt.astype(np.float16)
    xt8 = np.rint(xt * OUT_SCALE).astype(np.uint8)       # sa = 250*x in u8

    # softmax + gate-coeff collapse (O(out_dim*16) weight prep); the 250x
    # output scale and +2.5 offset fold into the c0 row (softmax weights
    # sum to 1), the 250x into cb; ca/cab stay raw because the gathered
    # sa = 250*x already carries the factor.
    w = np.exp(weight - weight.max(axis=-1, keepdims=True))
    w /= w.sum(axis=-1, keepdims=True)
    coeff = (w @ GATE_COEFFS).astype(np.float32)         # [out_dim, 4]
    coeff[:, 0] = OUT_SCALE * coeff[:, 0] + OUT_OFF
    coeff[:, 2] = OUT_SCALE * coeff[:, 2]

    in_maps = []
    for c in range(N_CORES):
        j0 = c * J_SHARD
        # idx[p, i] = a-row of chunk i, [p, 16+i] = b-row
        idx_c = np.empty((128, 2 * N_CHUNKS), dtype=np.int32)
        for i in range(N_CHUNKS):
            r = slice(j0 + i * CHUNK, j0 + (i + 1) * CHUNK)
            idx_c[:, i] = indices[0, r]
            idx_c[:, N_CHUNKS + i] = indices[1, r]
        # cct[p, 16k+i] = coeff k of output row j0 + 128i + p
        cc = coeff[j0:j0 + J_SHARD].reshape(N_CHUNKS, CHUNK, 4)
        cct = np.ascontiguousarray(
            cc.transpose(1, 2, 0).reshape(128, 4 * N_CHUNKS))
        in_maps.append({
            "xt16": xt16,
            "xt8": xt8,
            "idx": idx_c,
            "cct": cct,
        })

    trace = bool(os.environ.get("KERNEL_TRACE"))
    res = run_bass_kernel_spmd(nc, in_maps, core_ids=list(range(N_CORES)),
                               trace=trace)
    LAST_RESULT = res

    shards = [res.results[c]["out"] for c in range(N_CORES)]
    full = np.concatenate(shards, axis=0)                # [out_dim, batch] u8
    dec = (full.T.astype(np.float32) - OUT_OFF) * (1.0 / OUT_SCALE)
    return np.ascontiguousarray(dec)


# revision 48
# speedup vs baseline: 1.2034x; 1.2034x over previous
"""Trainium2 Bass kernel for nn_LogicDense (difflogic dense layer).

Math (reference):
    w      = softmax(weight, axis=-1)            # [out_dim, 16]
    coeffs = w @ GATE_COEFFS                     # [out_dim, 4] = (c0, ca, cb, cab)
    a      = x[:, indices[0]]                    # [batch, out_dim]
    b      = x[:, indices[1]]
    out    = c0 + ca*a + cb*b + cab*a*b          # [batch, out_dim]

Strategy (8 NeuronCores, tensor-parallel over out_dim):
    - Core c owns output rows j in [2048*c, 2048*(c+1)), processed in 16
      chunks of 128 rows (one output row per SBUF partition).
    - Host preps: x_t [in_dim, batch] as fp16 and as u8 (sa = round(250*x));
      per-core chunk indices [128, 16] i32; softmax+gate-coeff collapse
      (O(out_dim*16) weight prep) folded into a [128, 64] coeff tile.
    - Per chunk: two DGE indirect DMAs (indirect_dma_start, per-partition
      row offsets -- no Q7 ucode library, so no ~10us library-load stall
      and near-zero issue cost) gather a-rows (u8) and b-rows (fp16).
    - The DMA engines drain the qPoolDynamic ring FIFO, so gather issue is
      PACED to DEPTH chunks in flight: without it the DGE prefills the
      ring, the interleaved stores land tens of us late, and compute
      stalls on o-buf reuse (that failure mode measured +30us). Stores
      lag issue by ST_LAG chunks and NO=7 o-bufs absorb the remaining
      store-completion latency.
    - All 16 gates map [0,1]^2 -> [0,1] and softmax weights are convex, so
      out is in [0,1]. Computed as out_u = 250*out + 2.5 (in [2.5, 252.5])
      and the store DMA (gpsimd -- only SWDGE casts) converts fp16 -> u8,
      halving store traffic. Host decodes (u - 2.5)/250.
    - Per-partition coeff scalars give a 4-instruction combine:
         t = cab*sa + 250*cb           (tensor_scalar / ACT Identity)
         h = ca*sa + (250*c0+2.5)      (ACT Identity, scale/bias APs)
         o = t * b                     (DVE tensor_mul, fp16 2x)
         o = o + h                     (DVE tensor_add, fp16 2x)
      The ts op alternates ACT/DVE per chunk to balance engines
      (DVE ~90us, ACT ~90us busy; DMA ~97us busy at ~345 GB/s is the
      roofline). A dummy ACT op hoists the act-table load off the first
      data-dependent activation.
    - HBM/core: 8 MiB a(u8) + 16 MiB b(fp16) + 8 MiB store(u8) = 32 MiB;
      b must stay fp16: DVE tensor_tensor only reaches 2x with 2-byte
      operands, and a u8 mul at 1x would make DVE the bottleneck.
"""

import os
import sys

import numpy as np

sys.path.insert(0, "/opt/trn_rl_repo")

BATCH = 4096
IN_DIM = 8192
OUT_DIM = 16384
N_CORES = 8
J_SHARD = OUT_DIM // N_CORES        # 2048 output rows per core
CHUNK = 128                         # output rows per pipeline iteration
N_CHUNKS = J_SHARD // CHUNK         # 16
NG = 6                              # gather buffer slots (a and b paired)
NT = 2                              # t buffer sets (ts -> mul lifetime)
NH = 3                              # h buffer sets (h -> add lifetime)
NO = 7                              # output buffer sets (add -> store)
ST_LAG = 6                          # store issue lag behind gather issue
DEPTH = 4                           # max gather chunks in the DMA ring

OUT_SCALE = 250.0                   # out_u = OUT_SCALE*out + OUT_OFF
OUT_OFF = 2.5

GATE_COEFFS = np.array([
    [0, 0, 0, 0], [0, 0, 0, 1], [0, 1, 0, -1], [0, 1, 0, 0],
    [0, 0, 1, -1], [0, 0, 1, 0], [0, 1, 1, -2], [0, 1, 1, -1],
    [1, -1, -1, 1], [1, -1, -1, 2], [1, 0, -1, 0], [1, 0, -1, 1],
    [1, -1, 0, 0], [1, -1, 0, 1], [1, 0, 0, -1], [1, 0, 0, 0],
], dtype=np.float32)                # [16 gates, 4 bilinear coeffs]

_CACHE = {}
LAST_RESULT = None  # BassKernelResults of the most recent run (for profiling)


def _build_program():
    import concourse.bacc as bacc
    import concourse.bass as bass
    import concourse.mybir as mybir
    from contextlib import ExitStack

    dt = mybir.dt
    AF = mybir.ActivationFunctionType

    nc = bacc.Bacc("TRN2", target_bir_lowering=False, debug=False)

    xt16 = nc.dram_tensor("xt16", [IN_DIM, BATCH], dt.float16,
                          kind="ExternalInput")
    xt8 = nc.dram_tensor("xt8", [IN_DIM, BATCH], dt.uint8,
                         kind="ExternalInput")
    # cols [0,16): a-row index for chunk i; [16,32): b-row index
    idx = nc.dram_tensor("idx", [128, 2 * N_CHUNKS], dt.int32,
                         kind="ExternalInput")
    # [:, 16*k + i] = coeff k (0=c0,1=ca,2=cb,3=cab) for chunk i
    cct = nc.dram_tensor("cct", [128, 4 * N_CHUNKS], dt.float32,
                         kind="ExternalInput")
    out = nc.dram_tensor("out", [J_SHARD, BATCH], dt.uint8,
                         kind="ExternalOutput")

    with ExitStack() as ctx:
        sb = lambda name, shape, dty: ctx.enter_context(
            nc.sbuf_tensor(name, shape, dty))
        sb_idx = sb("sb_idx", [128, 2 * N_CHUNKS], dt.int32)
        sb_cc = sb("sb_cc", [128, 4 * N_CHUNKS], dt.float32)
        a_bufs = [sb(f"a{k}", [128, BATCH], dt.uint8) for k in range(NG)]
        b_bufs = [sb(f"b{k}", [128, BATCH], dt.float16) for k in range(NG)]
        t_bufs = [sb(f"t{k}", [128, BATCH], dt.float16) for k in range(NT)]
        h_bufs = [sb(f"h{k}", [128, BATCH], dt.float16) for k in range(NH)]
        o_bufs = [sb(f"o{k}", [128, BATCH], dt.float16) for k in range(NO)]

        # ts(i): even chunks on ACT, odd on DVE (tensor_scalar, u8 2x_2P).
        ts_on_act = lambda i: i % 2 == 0

        ops_act = []
        for i in range(N_CHUNKS):
            if ts_on_act(i):
                ops_act.append(('t', i))
            ops_act.append(('h', i))
        act_val = {op: n + 1 for n, op in enumerate(ops_act)}

        ops_dve = []
        for i in range(N_CHUNKS):
            if not ts_on_act(i):
                ops_dve.append(('ts', i))
            if i > 0:
                ops_dve.append(('add', i - 1))
            ops_dve.append(('mul', i))
        ops_dve.append(('add', N_CHUNKS - 1))
        dve_val = {op: n + 1 for n, op in enumerate(ops_dve)}

        sem = lambda name: ctx.enter_context(nc.semaphore(name))
        s_pi = sem("s_pi")
        s_pc = sem("s_pc")
        # slot k = chunk i % NG; a-gather of chunk i incs s_a[k] by 16,
        # b-gather incs s_g[k] by 16 (completion order between the two
        # DMAs is NOT guaranteed, hence separate semaphores).
        s_g = [sem(f"s_g{k}") for k in range(NG)]
        s_a = [sem(f"s_a{k}") for k in range(NG)]
        s_st = [sem(f"s_st{k}") for k in range(NO)]
        s_act = sem("s_act")
        s_dve = sem("s_dve")

        with nc.Block() as block:

            def cseg(k, i):  # per-partition scalar AP: coeff k, chunk i
                return sb_cc[:, 16 * k + i : 16 * k + i + 1]

            @block.sync
            def _(sync):
                sync.dma_start(sb_cc[:, :], cct[:, :]).then_inc(s_pc, 16)

            @block.gpsimd
            def _(gp):
                # load idx from the gp queue itself: gp starts ~2us before
                # the sync engine's first DMA reaches the queues.
                gp.dma_start(sb_idx[:, :], idx[:, :]).then_inc(s_pi, 16)
                gp.wait_ge(s_pi, 16)  # idx tile loaded

                def igather(dst, src, col):
                    # single_packet matches dma_gather's descriptor chaining
                    # (fewer per-packet overheads on the DMA engines).
                    inst = gp.indirect_dma_start(
                        out=dst, out_offset=None,
                        in_=src,
                        in_offset=bass.IndirectOffsetOnAxis(
                            ap=sb_idx[:, col:col + 1], axis=0),
                    )
                    inst.ins.single_packet = True
                    return inst

                def store(i):  # store chunk i (SWDGE cast fp16 -> u8)
                    ko = i % NO
                    gp.wait_ge(s_dve, dve_val[('add', i)])
                    if i >= NO:
                        gp.wait_ge(s_st[ko], 16 * (i // NO))
                    gp.dma_start(out[i * CHUNK:(i + 1) * CHUNK, :],
                                 o_bufs[ko][:, :]).then_inc(s_st[ko], 16)

                for i in range(N_CHUNKS):
                    k = i % NG
                    if i >= DEPTH:
                        # ring pacing: the DMA engines drain descriptors
                        # FIFO, so cap the gather backlog at DEPTH chunks --
                        # otherwise interleaved stores sit behind tens of us
                        # of gather bytes and compute stalls on o-buf reuse.
                        kp = (i - DEPTH) % NG
                        gp.wait_ge(s_g[kp], 16 * ((i - DEPTH) // NG + 1))
                    if i >= NG:
                        last = i - NG
                        # slot free once ts+h+mul of chunk last consumed it
                        # (mul(last) implies ts(last) on either engine).
                        gp.wait_ge(s_act, act_val[('h', last)])
                        gp.wait_ge(s_dve, dve_val[('mul', last)])
                        gp.wait_ge(s_a[k], 16 * (i // NG))
                        gp.wait_ge(s_g[k], 16 * (i // NG))
                    igather(a_bufs[k][:, :], xt8[:, :], i
                            ).then_inc(s_a[k], 16)
                    igather(b_bufs[k][:, :], xt16[:, :], N_CHUNKS + i
                            ).then_inc(s_g[k], 16)
                    # stores lag gather issue by ST_LAG chunks; add(i-ST_LAG)
                    # completes before gather i issues (compute trails the
                    # paced gather stream), so the gp queue never stalls.
                    if i >= ST_LAG:
                        store(i - ST_LAG)
                for i in range(N_CHUNKS - ST_LAG, N_CHUNKS):
                    store(i)
                for ko in range(NO):
                    if ko <= N_CHUNKS - 1:
                        n_st = (N_CHUNKS - 1 - ko) // NO + 1
                        gp.wait_ge(s_st[ko], 16 * n_st)

            @block.scalar
            def _(sc):
                sc.wait_ge(s_pc, 16)  # coeff tile ready
                # warmup op: hoists the auto-inserted ACT_TABLE_LOAD off the
                # first real activation (which waits on gather data).
                sc.activation(t_bufs[0][:, :1], sb_cc[:, :1], AF.Identity)
                for kind, i in ops_act:
                    ka = i % NG
                    sc.wait_ge(s_a[ka], 16 * (i // NG + 1))  # a(i) landed
                    if kind == 't':
                        kt = i % NT
                        # t[kt] free once DVE mul of i-NT consumed it
                        if i >= NT:
                            sc.wait_ge(s_dve, dve_val[('mul', i - NT)])
                        # t = cab*sa + 250*cb  (u8 source read directly)
                        sc.activation(t_bufs[kt][:, :], a_bufs[ka][:, :],
                                      AF.Identity,
                                      bias=cseg(2, i), scale=cseg(3, i),
                                      ).then_inc(s_act, 1)
                    else:
                        kh = i % NH
                        # h[kh] free once DVE add of i-NH completed
                        if i >= NH:
                            sc.wait_ge(s_dve, dve_val[('add', i - NH)])
                        # h = ca*sa + (250*c0 + 2.5)
                        sc.activation(h_bufs[kh][:, :], a_bufs[ka][:, :],
                                      AF.Identity,
                                      bias=cseg(0, i), scale=cseg(1, i),
                                      ).then_inc(s_act, 1)

            @block.vector
            def _(v):
                MU, AD = mybir.AluOpType.mult, mybir.AluOpType.add
                v.wait_ge(s_pc, 16)  # coeff tile ready
                for kind, i in ops_dve:
                    kt, kh = i % NT, i % NH
                    ko = i % NO
                    ka = i % NG
                    if kind == 'ts':
                        # t = (sa * cab) + 250*cb  (u8 src, 2x_2P mode)
                        v.wait_ge(s_a[ka], 16 * (i // NG + 1))
                        if i >= NT:
                            # t[kt] free once mul of chunk i-NT consumed it
                            v.wait_ge(s_dve, dve_val[('mul', i - NT)])
                        v.tensor_scalar(t_bufs[kt][:, :], a_bufs[ka][:, :],
                                        cseg(3, i), cseg(2, i), MU, AD,
                                        ).then_inc(s_dve, 1)
                    elif kind == 'mul':
                        v.wait_ge(s_g[ka], 16 * (i // NG + 1))  # b(i) landed
                        if ts_on_act(i):
                            v.wait_ge(s_act, act_val[('t', i)])
                        else:
                            v.wait_ge(s_dve, dve_val[('ts', i)])
                        if i >= NO:
                            # o[ko] free once store of chunk i-NO completed
                            v.wait_ge(s_st[ko], 16 * (i // NO))
                        v.tensor_mul(o_bufs[ko][:, :], t_bufs[kt][:, :],
                                     b_bufs[ka][:, :]).then_inc(s_dve, 1)
                    else:  # add
                        v.wait_ge(s_act, act_val[('h', i)])
                        v.wait_ge(s_dve, dve_val[('mul', i)])
                        v.tensor_add(o_bufs[ko][:, :],
                                     o_bufs[ko][:, :],
                                     h_bufs[kh][:, :]).then_inc(s_dve, 1)

    nc.compile()
    return nc


def _get_program():
    if "nc" not in _CACHE:
        _CACHE["nc"] = _build_program()
    return _CACHE["nc"]


def kernel(x, weight, indices):
    global LAST_RESULT
    from concourse.bass_utils import run_bass_kernel_spmd

    x = np.asarray(x, dtype=np.float32)
    weight = np.asarray(weight, dtype=np.float32)
    indices = np.asarray(indices)

    nc = _get_program()

    xt = np.ascontiguousarray(x.T)                       # [in_dim, batch] f32
    xt16 = xt.astype(np.float16)
    xt8 = np.rint(xt * OUT_SCALE).astype(np.uint8)       # sa = 250*x in u8

    # softmax + gate-coeff collapse (O(out_dim*16) weight prep); the 250x
    # output scale and +2.5 offset fold into the c0 row (softmax weights
    # sum to 1), the 250x into cb; ca/cab stay raw because the gathered
    # sa = 250*x already carries the factor.
    w = np.exp(weight - weight.max(axis=-1, keepdims=True))
    w /= w.sum(axis=-1, keepdims=True)
    coeff = (w @ GATE_COEFFS).astype(np.float32)         # [out_dim, 4]
    coeff[:, 0] = OUT_SCALE * coeff[:, 0] + OUT_OFF
    coeff[:, 2] = OUT_SCALE * coeff[:, 2]

    in_maps = []
    for c in range(N_CORES):
        j0 = c * J_SHARD
        # idx[p, i] = a-row of chunk i, [p, 16+i] = b-row
        idx_c = np.empty((128, 2 * N_CHUNKS), dtype=np.int32)
        for i in range(N_CHUNKS):
            r = slice(j0 + i * CHUNK, j0 + (i + 1) * CHUNK)
            idx_c[:, i] = indices[0, r]
            idx_c[:, N_CHUNKS + i] = indices[1, r]
        # cct[p, 16k+i] = coeff k of output row j0 + 128i + p
        cc = coeff[j0:j0 + J_SHARD].reshape(N_CHUNKS, CHUNK, 4)
        cct = np.ascontiguousarray(
            cc.transpose(1, 2, 0).reshape(128, 4 * N_CHUNKS))
        in_maps.append({
            "xt16": xt16,
            "xt8": xt8,
            "idx": idx_c,
            "cct": cct,
        })

    trace = bool(os.environ.get("KERNEL_TRACE"))
    res = run_bass_kernel_spmd(nc, in_maps, core_ids=list(range(N_CORES)),
                               trace=trace)
    LAST_RESULT = res

    shards = [res.results[c]["out"] for c in range(N_CORES)]
    full = np.concatenate(shards, axis=0)                # [out_dim, batch] u8
    dec = (full.T.astype(np.float32) - OUT_OFF) * (1.0 / OUT_SCALE)
    return np.ascontiguousarray(dec)


# revision 49
# speedup vs baseline: 1.2247x; 1.0177x over previous
"""Trainium2 Bass kernel for nn_LogicDense (difflogic dense layer).

Math (reference):
    w      = softmax(weight, axis=-1)            # [out_dim, 16]
    coeffs = w @ GATE_COEFFS                     # [out_dim, 4] = (c0, ca, cb, cab)
    a      = x[:, indices[0]]                    # [batch, out_dim]
    b      = x[:, indices[1]]
    out    = c0 + ca*a + cb*b + cab*a*b          # [batch, out_dim]

Strategy (8 NeuronCores, tensor-parallel over out_dim):
    - Core c owns output rows j in [2048*c, 2048*(c+1)), processed in 16
      chunks of 128 rows (one output row per SBUF partition).
    - Host preps: x_t [in_dim, batch] as fp16 and as u8 (sa = round(250*x));
      per-core chunk indices [128, 16] i32; softmax+gate-coeff collapse
      (O(out_dim*16) weight prep) folded into a [128, 64] coeff tile.
    - Per chunk: two DGE indirect DMAs (indirect_dma_start, per-partition
      row offsets -- no Q7 ucode library, so no ~10us library-load stall
      and near-zero issue cost) gather a-rows (u8) and b-rows (fp16).
    - The DMA engines drain the qPoolDynamic ring FIFO, so gather issue is
      PACED to DEPTH chunks in flight: without it the DGE prefills the
      ring, the interleaved stores land tens of us late, and compute
      stalls on o-buf reuse (that failure mode measured +30us). Stores
      lag issue by ST_LAG chunks and NO=7 o-bufs absorb the remaining
      store-completion latency.
    - All 16 gates map [0,1]^2 -> [0,1] and softmax weights are convex, so
      out is in [0,1]. Computed as out_u = 250*out + 2.5 (in [2.5, 252.5])
      and the store DMA (gpsimd -- only SWDGE casts) converts fp16 -> u8,
      halving store traffic. Host decodes (u - 2.5)/250.
    - Per-partition coeff scalars give a 4-instruction combine:
         t = cab*sa + 250*cb           (tensor_scalar / ACT Identity)
         h = ca*sa + (250*c0+2.5)      (ACT Identity, scale/bias APs)
         o = t * b                     (DVE tensor_mul, fp16 2x)
         o = o + h                     (DVE tensor_add, fp16 2x)
      The ts op alternates ACT/DVE per chunk to balance engines
      (DVE ~90us, ACT ~90us busy; DMA ~97us busy at ~345 GB/s is the
      roofline). A dummy ACT op hoists the act-table load off the first
      data-dependent activation.
    - HBM/core: 8 MiB a(u8) + 16 MiB b(fp16) + 8 MiB store(u8) = 32 MiB;
      b must stay fp16: DVE tensor_tensor only reaches 2x with 2-byte
      operands, and a u8 mul at 1x would make DVE the bottleneck.
"""

import os
import sys

import numpy as np

sys.path.insert(0, "/opt/trn_rl_repo")

BATCH = 4096
IN_DIM = 8192
OUT_DIM = 16384
N_CORES = 8
J_SHARD = OUT_DIM // N_CORES        # 2048 output rows per core
CHUNK = 128                         # output rows per pipeline iteration
N_CHUNKS = J_SHARD // CHUNK         # 16
NG = 6                              # gather buffer slots (a and b paired)
NT = 2                              # t buffer sets (ts -> mul lifetime)
NH = 3                              # h buffer sets (h -> add lifetime)
NO = 7                              # output buffer sets (add -> store)
ST_LAG = 5                          # store issue lag behind gather issue
DEPTH = 4                           # max gather chunks in the DMA ring

OUT_SCALE = 250.0                   # out_u = OUT_SCALE*out + OUT_OFF
OUT_OFF = 2.5

GATE_COEFFS = np.array([
    [0, 0, 0, 0], [0, 0, 0, 1], [0, 1, 0, -1], [0, 1, 0, 0],
    [0, 0, 1, -1], [0, 0, 1, 0], [0, 1, 1, -2], [0, 1, 1, -1],
    [1, -1, -1, 1], [1, -1, -1, 2], [1, 0, -1, 0], [1, 0, -1, 1],
    [1, -1, 0, 0], [1, -1, 0, 1], [1, 0, 0, -1], [1, 0, 0, 0],
], dtype=np.float32)                # [16 gates, 4 bilinear coeffs]

_CACHE = {}
LAST_RESULT = None  # BassKernelResults of the most recent run (for profiling)


def _build_program():
    import concourse.bacc as bacc
    import concourse.bass as bass
    import concourse.mybir as mybir
    from contextlib import ExitStack

    dt = mybir.dt
    AF = mybir.ActivationFunctionType

    nc = bacc.Bacc("TRN2", target_bir_lowering=False, debug=False)

    xt16 = nc.dram_tensor("xt16", [IN_DIM, BATCH], dt.float16,
                          kind="ExternalInput")
    xt8 = nc.dram_tensor("xt8", [IN_DIM, BATCH], dt.uint8,
                         kind="ExternalInput")
    # cols [0,16): a-row index for chunk i; [16,32): b-row index
    idx = nc.dram_tensor("idx", [128, 2 * N_CHUNKS], dt.int32,
                         kind="ExternalInput")
    # [:, 16*k + i] = coeff k (0=c0,1=ca,2=cb,3=cab) for chunk i
    cct = nc.dram_tensor("cct", [128, 4 * N_CHUNKS], dt.float32,
                         kind="ExternalInput")
    out = nc.dram_tensor("out", [J_SHARD, BATCH], dt.uint8,
                         kind="ExternalOutput")

    with ExitStack() as ctx:
        sb = lambda name, shape, dty: ctx.enter_context(
            nc.sbuf_tensor(name, shape, dty))
        sb_idx = sb("sb_idx", [128, 2 * N_CHUNKS], dt.int32)
        sb_cc = sb("sb_cc", [128, 4 * N_CHUNKS], dt.float32)
        a_bufs = [sb(f"a{k}", [128, BATCH], dt.uint8) for k in range(NG)]
        b_bufs = [sb(f"b{k}", [128, BATCH], dt.float16) for k in range(NG)]
        t_bufs = [sb(f"t{k}", [128, BATCH], dt.float16) for k in range(NT)]
        h_bufs = [sb(f"h{k}", [128, BATCH], dt.float16) for k in range(NH)]
        o_bufs = [sb(f"o{k}", [128, BATCH], dt.float16) for k in range(NO)]

        # ts(i): even chunks on ACT, odd on DVE (tensor_scalar, u8 2x_2P).
        ts_on_act = lambda i: i % 2 == 0

        ops_act = []
        for i in range(N_CHUNKS):
            if ts_on_act(i):
                ops_act.append(('t', i))
            ops_act.append(('h', i))
        act_val = {op: n + 1 for n, op in enumerate(ops_act)}

        ops_dve = []
        for i in range(N_CHUNKS):
            if not ts_on_act(i):
                ops_dve.append(('ts', i))
            if i > 0:
                ops_dve.append(('add', i - 1))
            ops_dve.append(('mul', i))
        ops_dve.append(('add', N_CHUNKS - 1))
        dve_val = {op: n + 1 for n, op in enumerate(ops_dve)}

        sem = lambda name: ctx.enter_context(nc.semaphore(name))
        s_pi = sem("s_pi")
        s_pc = sem("s_pc")
        # slot k = chunk i % NG; a-gather of chunk i incs s_a[k] by 16,
        # b-gather incs s_g[k] by 16 (completion order between the two
        # DMAs is NOT guaranteed, hence separate semaphores).
        s_g = [sem(f"s_g{k}") for k in range(NG)]
        s_a = [sem(f"s_a{k}") for k in range(NG)]
        s_st = [sem(f"s_st{k}") for k in range(NO)]
        s_act = sem("s_act")
        s_dve = sem("s_dve")

        with nc.Block() as block:

            def cseg(k, i):  # per-partition scalar AP: coeff k, chunk i
                return sb_cc[:, 16 * k + i : 16 * k + i + 1]

            @block.sync
            def _(sync):
                sync.dma_start(sb_cc[:, :], cct[:, :]).then_inc(s_pc, 16)

            @block.gpsimd
            def _(gp):
                # load idx from the gp queue itself: gp starts ~2us before
                # the sync engine's first DMA reaches the queues.
                gp.dma_start(sb_idx[:, :], idx[:, :]).then_inc(s_pi, 16)
                gp.wait_ge(s_pi, 16)  # idx tile loaded

                def igather(dst, src, col):
                    # single_packet matches dma_gather's descriptor chaining
                    # (fewer per-packet overheads on the DMA engines).
                    inst = gp.indirect_dma_start(
                        out=dst, out_offset=None,
                        in_=src,
                        in_offset=bass.IndirectOffsetOnAxis(
                            ap=sb_idx[:, col:col + 1], axis=0),
                    )
                    inst.ins.single_packet = True
                    return inst

                def store(i):  # store chunk i (SWDGE cast fp16 -> u8)
                    ko = i % NO
                    gp.wait_ge(s_dve, dve_val[('add', i)])
                    if i >= NO:
                        gp.wait_ge(s_st[ko], 16 * (i // NO))
                    gp.dma_start(out[i * CHUNK:(i + 1) * CHUNK, :],
                                 o_bufs[ko][:, :]).then_inc(s_st[ko], 16)

                for i in range(N_CHUNKS):
                    k = i % NG
                    if i >= DEPTH:
                        # ring pacing: the DMA engines drain descriptors
                        # FIFO, so cap the gather backlog at DEPTH chunks --
                        # otherwise interleaved stores sit behind tens of us
                        # of gather bytes and compute stalls on o-buf reuse.
                        kp = (i - DEPTH) % NG
                        gp.wait_ge(s_g[kp], 16 * ((i - DEPTH) // NG + 1))
                    if i >= NG:
                        last = i - NG
                        # slot free once ts+h+mul of chunk last consumed it
                        # (mul(last) implies ts(last) on either engine).
                        gp.wait_ge(s_act, act_val[('h', last)])
                        gp.wait_ge(s_dve, dve_val[('mul', last)])
                        gp.wait_ge(s_a[k], 16 * (i // NG))
                        gp.wait_ge(s_g[k], 16 * (i // NG))
                    igather(a_bufs[k][:, :], xt8[:, :], i
                            ).then_inc(s_a[k], 16)
                    igather(b_bufs[k][:, :], xt16[:, :], N_CHUNKS + i
                            ).then_inc(s_g[k], 16)
                    # stores lag gather issue by ST_LAG chunks; add(i-ST_LAG)
                    # completes before gather i issues (compute trails the
                    # paced gather stream), so the gp queue never stalls.
                    if i >= ST_LAG:
                        store(i - ST_LAG)
                for i in range(N_CHUNKS - ST_LAG, N_CHUNKS):
                    store(i)
                for ko in range(NO):
                    if ko <= N_CHUNKS - 1:
                        n_st = (N_CHUNKS - 1 - ko) // NO + 1
                        gp.wait_ge(s_st[ko], 16 * n_st)

            @block.scalar
            def _(sc):
                sc.wait_ge(s_pc, 16)  # coeff tile ready
                # warmup op: hoists the auto-inserted ACT_TABLE_LOAD off the
                # first real activation (which waits on gather data).
                sc.activation(t_bufs[0][:, :1], sb_cc[:, :1], AF.Identity)
                for kind, i in ops_act:
                    ka = i % NG
                    sc.wait_ge(s_a[ka], 16 * (i // NG + 1))  # a(i) landed
                    if kind == 't':
                        kt = i % NT
                        # t[kt] free once DVE mul of i-NT consumed it
                        if i >= NT:
                            sc.wait_ge(s_dve, dve_val[('mul', i - NT)])
                        # t = cab*sa + 250*cb  (u8 source read directly)
                        sc.activation(t_bufs[kt][:, :], a_bufs[ka][:, :],
                                      AF.Identity,
                                      bias=cseg(2, i), scale=cseg(3, i),
                                      ).then_inc(s_act, 1)
                    else:
                        kh = i % NH
                        # h[kh] free once DVE add of i-NH completed
                        if i >= NH:
                            sc.wait_ge(s_dve, dve_val[('add', i - NH)])
                        # h = ca*sa + (250*c0 + 2.5)
                        sc.activation(h_bufs[kh][:, :], a_bufs[ka][:, :],
                                      AF.Identity,
                                      bias=cseg(0, i), scale=cseg(1, i),
                                      ).then_inc(s_act, 1)

            @block.vector
            def _(v):
                MU, AD = mybir.AluOpType.mult, mybir.AluOpType.add
                v.wait_ge(s_pc, 16)  # coeff tile ready
                for kind, i in ops_dve:
                    kt, kh = i % NT, i % NH
                    ko = i % NO
                    ka = i % NG
                    if kind == 'ts':
                        # t = (sa * cab) + 250*cb  (u8 src, 2x_2P mode)
                        v.wait_ge(s_a[ka], 16 * (i // NG + 1))
                        if i >= NT:
                            # t[kt] free once mul of chunk i-NT consumed it
                            v.wait_ge(s_dve, dve_val[('mul', i - NT)])
                        v.tensor_scalar(t_bufs[kt][:, :], a_bufs[ka][:, :],
                                        cseg(3, i), cseg(2, i), MU, AD,
                                        ).then_inc(s_dve, 1)
                    elif kind == 'mul':
                        v.wait_ge(s_g[ka], 16 * (i // NG + 1))  # b(i) landed
                        if ts_on_act(i):
                            v.wait_ge(s_act, act_val[('t', i)])
                        else:
                            v.wait_ge(s_dve, dve_val[('ts', i)])
                        if i >= NO:
                            # o[ko] free once store of chunk i-NO completed
                            v.wait_ge(s_st[ko], 16 * (i // NO))
                        v.tensor_mul(o_bufs[ko][:, :], t_bufs[kt][:, :],
                                     b_bufs[ka][:, :]).then_inc(s_dve, 1)
                    else:  # add
                        v.wait_ge(s_act, act_val[('h', i)])
                        v.wait_ge(s_dve, dve_val[('mul', i)])
                        v.tensor_add(o_bufs[ko][:, :],
                                     o_bufs[ko][:, :],
                                     h_bufs[kh][:, :]).then_inc(s_dve, 1)

    nc.compile()
    return nc


def _get_program():
    if "nc" not in _CACHE:
        _CACHE["nc"] = _build_program()
    return _CACHE["nc"]


def kernel(x, weight, indices):
    global LAST_RESULT
    from concourse.bass_utils import run_bass_kernel_spmd

    x = np.asarray(x, dtype=np.float32)
    weight = np.asarray(weight, dtype=np.float32)
    indices = np.asarray(indices)

    nc = _get_program()

    xt = np.ascontiguousarray(x.T)                       # [in_dim, batch] f32
    xt16 = xt.astype(np.float16)
    xt8 = np.rint(xt * OUT_SCALE).astype(np.uint8)       # sa = 250*x in u8

    # softmax + gate-coeff collapse (O(out_dim*16) weight prep); the 250x
    # output scale and +2.5 offset fold into the c0 row (softmax weights
    # sum to 1), the 250x into cb; ca/cab stay raw because the gathered
    # sa = 250*x already carries the factor.
    w = np.exp(weight - weight.max(axis=-1, keepdims=True))
    w /= w.sum(axis=-1, keepdims=True)
    coeff = (w @ GATE_COEFFS).astype(np.float32)         # [out_dim, 4]
    coeff[:, 0] = OUT_SCALE * coeff[:, 0] + OUT_OFF
    coeff[:, 2] = OUT_SCALE * coeff[:, 2]

    in_maps = []
    for c in range(N_CORES):
        j0 = c * J_SHARD
        # idx[p, i] = a-row of chunk i, [p, 16+i] = b-row
        idx_c = np.empty((128, 2 * N_CHUNKS), dtype=np.int32)
        for i in range(N_CHUNKS):
            r = slice(j0 + i * CHUNK, j0 + (i + 1) * CHUNK)
            idx_c[:, i] = indices[0, r]
            idx_c[:, N_CHUNKS + i] = indices[1, r]
        # cct[p, 16k+i] = coeff k of output row j0 + 128i + p
        cc = coeff[j0:j0 + J_SHARD].reshape(N_CHUNKS, CHUNK, 4)
        cct = np.ascontiguousarray(
            cc.transpose(1, 2, 0).reshape(128, 4 * N_CHUNKS))
        in_maps.append({
            "xt16": xt16,
            "xt8": xt8,
            "idx": idx_c,
            "cct": cct,
        })

    trace = bool(os.environ.get("KERNEL_TRACE"))
    res = run_bass_kernel_spmd(nc, in_maps, core_ids=list(range(N_CORES)),
                               trace=trace)
    LAST_RESULT = res

    shards = [res.results[c]["out"] for c in range(N_CORES)]
    full = np.concatenate(shards, axis=0)                # [out_dim, batch] u8
    dec = (full.T.astype(np.float32) - OUT_OFF) * (1.0 / OUT_SCALE)
    return np.ascontiguousarray(dec)
